# revision 7
# baseline (speedup 1.0000x reference)
"""Causal self-attention (nn_CausalSelfAttention) on 8 TRN2 NeuronCores.

Reference computation (B=2, T=2048, C=1024, H=16 heads, D=64):
    qkv = x @ W_attn.T + b_attn ; split q,k,v
    y   = softmax(causal(q k^T / sqrt(D))) v        (per head)
    out = y @ W_proj.T + b_proj

Sharding: batch (2-way) x head-group (4-way, 4 heads each) -> 8 cores.
Each core computes its batch's attention for its 4 heads plus the partial
c_proj contribution of those heads' channels; the host sums the 4 partials
per batch and adds the (adjusted) bias once.

Device-side simplifications (exact up to fp error):
  - k bias dropped: it shifts every score in a softmax row by the same
    constant, which cancels.
  - v bias folded into the host-side output bias: sum(P)=1 per row, so
    y = P v + bv and the bv term becomes W_p @ bv_full added once on host.

Per-core kernel (strip-pipelined, strips of 512 queries, s=0..3):
    per strip: k tiles [128,128] (2 heads stacked), q [128,512] (+bias via
    ACT Identity), v tiles [128,256]; then per 128-key round:
      S^T pair [128,1024] = two row-packed K=64 matmuls (heads at array
        rows 0-63 / 64-127) into a 2-bank PSUM tile
      P^T pair = one ACT Exp over the pair tile -> SBUF bf16
        (causal diagonal blocks masked via gpsimd affine_select)
      PV: col-packed pair (M=64+64 into one bank) accumulated over rounds
      denominators: 4 concurrent M=1 matmuls with a ones column into
        PSUM partitions {0,32,64,96}
    normalize via DVE reciprocal + gpsimd partition_broadcast + DVE mult
    projection emits out^T = (wp tile).T @ y strip; host transposes back.
"""
import math
from contextlib import ExitStack

import ml_dtypes
import numpy as np

import concourse.bacc as bacc
import concourse.bass as bass
import concourse.mybir as mybir
import concourse.tile as tile
from concourse.bass_utils import run_bass_kernel_spmd

F32 = mybir.dt.float32
BF16 = mybir.dt.bfloat16
MMDT = BF16                    # dtype for all TensorE-facing tensors

N_CORES = 8
B, T, C, H = 2, 2048, 1024, 16
D = 64
GROUPS = N_CORES // B          # head groups per batch = 4
HPC = H // GROUPS              # heads per core = 4
CS = HPC * D                   # channel slice per core = 256
KT = C // 128                  # contraction tiles over C = 8
NS = T // 512                  # 512-wide query strips = 4
TT = T // 128                  # 128-row key tiles = 16


def build_nc():
    nc = bacc.Bacc("TRN2", target_bir_lowering=False, debug=False,
                   num_devices=N_CORES)

    xT = nc.dram_tensor("xT", [C, T], MMDT, kind="ExternalInput")
    wqkT = nc.dram_tensor("wqkT", [C, 2 * CS], MMDT, kind="ExternalInput")
    bq = nc.dram_tensor("bq", [2, 128, 1], F32, kind="ExternalInput")
    wvT = nc.dram_tensor("wvT", [C, CS], MMDT, kind="ExternalInput")
    wpT = nc.dram_tensor("wpT", [CS, C], MMDT, kind="ExternalInput")
    outT = nc.dram_tensor("outT", [C, T], F32, kind="ExternalOutput")

    xTr = xT.ap().rearrange("(kt p) t -> kt p t", p=128)
    wqkr = wqkT.ap().rearrange("(kt p) n -> kt p n", p=128)
    wvr = wvT.ap().rearrange("(kt p) n -> kt p n", p=128)
    wpr = wpT.ap().rearrange("(kt p) n -> kt p n", p=128)

    scale = 1.0 / math.sqrt(D)

    with tile.TileContext(nc) as tc, ExitStack() as ctx:
        pw = ctx.enter_context(tc.tile_pool(name="pw", bufs=1))
        px = ctx.enter_context(tc.tile_pool(name="px", bufs=1))
        pq = ctx.enter_context(tc.tile_pool(name="pq", bufs=1))
        pk = ctx.enter_context(tc.tile_pool(name="pk", bufs=1))
        pv = ctx.enter_context(tc.tile_pool(name="pv", bufs=1))
        py = ctx.enter_context(tc.tile_pool(name="py", bufs=1))
        ppt = ctx.enter_context(tc.tile_pool(name="ppt", bufs=4))
        pnorm = ctx.enter_context(tc.tile_pool(name="pnorm", bufs=3))
        pout = ctx.enter_context(tc.tile_pool(name="pout", bufs=4))
        # PSUM: 8 banks exactly: sq 2x2 + pv 2x1 + dn 1 + pm 1
        psq = ctx.enter_context(tc.tile_pool(name="psq", bufs=2, space="PSUM"))
        ppv = ctx.enter_context(tc.tile_pool(name="ppv", bufs=2, space="PSUM"))
        pdn = ctx.enter_context(tc.tile_pool(name="pdn", bufs=1, space="PSUM"))
        ppm = ctx.enter_context(tc.tile_pool(name="ppm", bufs=1, space="PSUM"))

        # ---- input DMA ----
        wqk_sb, wv_sb = [], []
        for k in range(KT):
            wt = pw.tile([128, 2 * CS], MMDT, tag=f"wqk{k}", name=f"wqk{k}")
            nc.gpsimd.dma_start(wt[:], wqkr[k])
            wqk_sb.append(wt)
        for k in range(KT):
            vt = pw.tile([128, CS], MMDT, tag=f"wv{k}", name=f"wv{k}")
            nc.gpsimd.dma_start(vt[:], wvr[k])
            wv_sb.append(vt)
        bq_sb = []
        for m in range(2):
            bt = pw.tile([128, 1], F32, tag=f"bq{m}", name=f"bq{m}")
            nc.gpsimd.dma_start(bt[:], bq.ap()[m])
            bq_sb.append(bt)
        wp_sb = []
        for k2 in range(2):
            pt_ = pw.tile([128, C], MMDT, tag=f"wp{k2}", name=f"wp{k2}")
            nc.gpsimd.dma_start(pt_[:], wpr[k2])
            wp_sb.append(pt_)
        # x: strips 0,1 as [128,512] quarters (early start), 2,3 as halves
        xq = [[None] * 2 for _ in range(KT)]   # [k][s] s in 0,1
        xh = [None] * KT                        # [k] cols 1024:2048
        for k in range(KT):
            for s in range(2):
                t_ = px.tile([128, 512], MMDT, tag=f"xq{k}_{s}",
                             name=f"xq{k}_{s}")
                nc.sync.dma_start(t_[:], xTr[k][:, s * 512:(s + 1) * 512])
                xq[k][s] = t_
        for k in range(KT):
            t_ = px.tile([128, 1024], MMDT, tag=f"xh{k}", name=f"xh{k}")
            nc.scalar.dma_start(t_[:], xTr[k][:, 1024:2048])
            xh[k] = t_
        ones_sb = pw.tile([128, 4], MMDT, tag="ones", name="ones")
        nc.vector.memset(ones_sb[:], 1.0)
        zlhs = pw.tile([1, 128], MMDT, tag="zlhs", name="zlhs")
        nc.vector.memset(zlhs[:], 0.0)
        zrow = pw.tile([1, 512], MMDT, tag="zrow", name="zrow")
        nc.vector.memset(zrow[:], 0.0)

        def x_strip(k, s):
            """AP of x columns [s*512, (s+1)*512) for contraction tile k."""
            if s < 2:
                return xq[k][s][:]
            return xh[k][:, (s - 2) * 512:(s - 1) * 512]

        # persistent SBUF tensors
        q_sb = [pq.tile([128, T], MMDT, tag=f"q{m}", name=f"q{m}")
                for m in range(2)]
        k_sb = [[pk.tile([128, 128], MMDT, tag=f"k{mp}_{n}", name=f"k{mp}_{n}")
                 for n in range(TT)] for mp in range(2)]
        v_sb = [pv.tile([128, CS], MMDT, tag=f"v{n}", name=f"v{n}")
                for n in range(TT)]
        y_sb = [py.tile([128, T], MMDT, tag=f"y{k2}", name=f"y{k2}")
                for k2 in range(2)]

        for s in range(NS):
            # ---- k production: pair mp holds heads (2mp, 2mp+1) ----
            for mp in range(2):
                ps = psq.tile([128, 1024], F32, tag="sq", name="ps_k")
                for k in range(KT):
                    nc.tensor.matmul(
                        ps[:, 0:512],
                        wqk_sb[k][:, (2 + mp) * 128:(3 + mp) * 128],
                        x_strip(k, s),
                        start=(k == 0), stop=(k == KT - 1),
                    )
                for j in range(4):
                    nc.vector.tensor_copy(k_sb[mp][4 * s + j][:],
                                          ps[:, j * 128:(j + 1) * 128])
            # ---- q production (bias via ACT Identity) ----
            for mp in range(2):
                ps = psq.tile([128, 1024], F32, tag="sq", name="ps_q")
                for k in range(KT):
                    nc.tensor.matmul(
                        ps[:, 0:512],
                        wqk_sb[k][:, mp * 128:(mp + 1) * 128],
                        x_strip(k, s),
                        start=(k == 0), stop=(k == KT - 1),
                    )
                nc.scalar.activation(
                    q_sb[mp][:, s * 512:(s + 1) * 512], ps[:, 0:512],
                    mybir.ActivationFunctionType.Identity, bias=bq_sb[mp][:])
            # ---- v production for key tiles 4s..4s+3 ----
            for j in range(4):
                n = 4 * s + j
                ps = ppm.tile([128, 512], F32, tag="pm", name="ps_v")
                for k in range(KT):
                    nc.tensor.matmul(
                        ps[:, 0:CS],
                        x_strip(k, s)[:, j * 128:(j + 1) * 128],
                        wv_sb[k][:],
                        start=(k == 0), stop=(k == KT - 1),
                    )
                nc.vector.tensor_copy(v_sb[n][:], ps[:, 0:CS])

        for s in range(NS):
            # ---- attention for strip s ----
            nt = 4 * s + 4
            pv_ps = [ppv.tile([128, 512], F32, tag="pv", name=f"pv{pp}")
                     for pp in range(2)]
            dn = pdn.tile([128, 512], F32, tag="dn", name="dn")
            # open one accumulation chain per bank, zero-filling the whole
            # bank so later partial writes accumulate onto zeros
            for pp in range(2):
                nc.tensor.matmul(pv_ps[pp][:], zlhs[:], zrow[:],
                                 start=True, stop=False)
            nc.tensor.matmul(dn[:], zlhs[:], zrow[:], start=True, stop=False)
            for n in range(nt):
                off = max(0, n - 4 * s) * 128
                pts = []
                for pp in range(2):
                    st = psq.tile([128, 1024], F32, tag="sq", name="st")
                    for r in range(2):
                        nc.tensor.matmul(
                            st[:, r * 512 + off:(r + 1) * 512],
                            k_sb[pp][n][r * 64:(r + 1) * 64, :],
                            q_sb[pp][r * 64:(r + 1) * 64,
                                     s * 512 + off:(s + 1) * 512],
                            start=True, stop=True,
                        )
                    pt = ppt.tile([128, 1024], MMDT, tag="pt", name="pt")
                    if off == 0:
                        nc.scalar.activation(
                            pt[:], st[:],
                            mybir.ActivationFunctionType.Exp, scale=scale)
                    else:
                        for r in range(2):
                            nc.scalar.activation(
                                pt[:, r * 512 + off:(r + 1) * 512],
                                st[:, r * 512 + off:(r + 1) * 512],
                                mybir.ActivationFunctionType.Exp, scale=scale)
                    if n >= 4 * s:
                        # mixed diagonal block: keep where query >= key
                        for r in range(2):
                            nc.gpsimd.affine_select(
                                out=pt[:, r * 512 + off:r * 512 + off + 128],
                                in_=pt[:, r * 512 + off:r * 512 + off + 128],
                                compare_op=mybir.AluOpType.is_ge,
                                fill=0.0, base=0,
                                pattern=[[1, 128]], channel_multiplier=-1)
                    pts.append(pt)
                for pp in range(2):
                    for r in range(2):
                        # one accumulation chain per PSUM bank: start marks
                        # the whole bank pending-zero, so only the first MM
                        # starts and only the last stops
                        nc.tensor.matmul(
                            pv_ps[pp][r * 64:(r + 1) * 64, off:512],
                            v_sb[n][:, (2 * pp + r) * 64:(2 * pp + r + 1) * 64],
                            pts[pp][:, r * 512 + off:(r + 1) * 512],
                            start=False,
                            stop=(n == nt - 1 and r == 1),
                        )
                for pp in range(2):
                    for r in range(2):
                        h4 = 2 * pp + r
                        nc.tensor.matmul(
                            dn[32 * h4:32 * h4 + 1, off:512],
                            ones_sb[:, h4:h4 + 1],
                            pts[pp][:, r * 512 + off:(r + 1) * 512],
                            start=False,
                            stop=(n == nt - 1 and h4 == 3),
                            tile_position=(0, 32 * h4),
                        )

            # ---- normalize: y = y_unnorm / denom ----
            for pp in range(2):
                for r in range(2):
                    h4 = 2 * pp + r
                    # custom DVE/gpsimd ops read partition 0 only: stage the
                    # denominator row at base partition 0 before recip/bcast
                    dtmp = pnorm.tile([1, 512], F32, tag="dtmp", name="dtmp")
                    nc.vector.tensor_copy(dtmp[:], dn[32 * h4:32 * h4 + 1, :])
                    rr = pnorm.tile([1, 512], F32, tag="rr", name="rr")
                    nc.vector.reciprocal_approx_fast(rr[:], dtmp[:])
                    rb = pnorm.tile([64, 512], F32, tag="rb", name="rb")
                    nc.gpsimd.partition_broadcast(rb[:], rr[:])
                    nc.vector.tensor_tensor(
                        y_sb[pp][r * 64:(r + 1) * 64, s * 512:(s + 1) * 512],
                        pv_ps[pp][r * 64:(r + 1) * 64, :], rb[:],
                        op=mybir.AluOpType.mult)

            # ---- projection for this strip: out^T [C, 512] ----
            for ct in range(8):
                ps = ppm.tile([128, 512], F32, tag="pm", name="ps_o")
                for k2 in range(2):
                    nc.tensor.matmul(
                        ps[:],
                        wp_sb[k2][:, ct * 128:(ct + 1) * 128],
                        y_sb[k2][:, s * 512:(s + 1) * 512],
                        start=(k2 == 0), stop=(k2 == 1),
                    )
                ot = pout.tile([128, 512], F32, tag="ot", name="ot")
                nc.vector.tensor_copy(ot[:], ps[:])
                nc.sync.dma_start(
                    outT.ap()[ct * 128:(ct + 1) * 128,
                              s * 512:(s + 1) * 512],
                    ot[:])

    nc.compile()
    return nc


def make_in_maps(x, W_attn, b_attn, W_proj):
    """Shard full inputs into the 8 per-core input dicts."""
    x = np.asarray(x, dtype=np.float32)
    W_attn = np.asarray(W_attn, dtype=np.float32)
    b_attn = np.asarray(b_attn, dtype=np.float32)
    W_proj = np.asarray(W_proj, dtype=np.float32)
    in_maps = []
    xTb = [np.ascontiguousarray(x[b_].T) for b_ in range(B)]
    for core in range(N_CORES):
        b_ = core // GROUPS
        g = core % GROUPS
        sl = slice(g * CS, (g + 1) * CS)
        wq = W_attn[sl, :]
        wk = W_attn[C + g * CS:C + (g + 1) * CS, :]
        wv = W_attn[2 * C + g * CS:2 * C + (g + 1) * CS, :]
        bqs = b_attn[sl]
        in_maps.append({
            "xT": xTb[b_].astype(ml_dtypes.bfloat16),
            "wqkT": np.ascontiguousarray(
                np.concatenate([wq, wk], 0).T).astype(ml_dtypes.bfloat16),
            "bq": np.ascontiguousarray(bqs.reshape(2, 128, 1)),
            "wvT": np.ascontiguousarray(wv.T).astype(ml_dtypes.bfloat16),
            "wpT": np.ascontiguousarray(
                W_proj[:, g * CS:(g + 1) * CS].T).astype(ml_dtypes.bfloat16),
        })
    return in_maps


_NC = None


def _get_nc():
    global _NC
    if _NC is None:
        _NC = build_nc()
    return _NC


def run(x, W_attn, b_attn, W_proj, b_proj, trace=False):
    nc = _get_nc()
    in_maps = make_in_maps(x, W_attn, b_attn, W_proj)
    res = run_bass_kernel_spmd(nc, in_maps, core_ids=list(range(N_CORES)),
                               trace=trace)
    out = np.zeros((B, T, C), dtype=np.float32)
    for core in range(N_CORES):
        out[core // GROUPS] += res.results[core]["outT"].T
    # b_proj plus the folded-in v bias: y = P v + bv, sum(P)=1 per row
    b_eff = (np.asarray(b_proj, dtype=np.float32)
             + np.asarray(W_proj, dtype=np.float32)
             @ np.asarray(b_attn, dtype=np.float32)[2 * C:3 * C])
    out += b_eff[None, None, :]
    return out, res


def kernel(x, W_attn, b_attn, W_proj, b_proj):
    out, _ = run(x, W_attn, b_attn, W_proj, b_proj, trace=False)
    return out


# revision 8
# speedup vs baseline: 1.0632x; 1.0632x over previous
"""Causal self-attention (nn_CausalSelfAttention) on 8 TRN2 NeuronCores.

Reference computation (B=2, T=2048, C=1024, H=16 heads, D=64):
    qkv = x @ W_attn.T + b_attn ; split q,k,v
    y   = softmax(causal(q k^T / sqrt(D))) v        (per head)
    out = y @ W_proj.T + b_proj

Sharding: batch (2-way) x head-group (4-way, 4 heads each) -> 8 cores.
Each core computes its batch's attention for its 4 heads plus the partial
c_proj contribution of those heads' channels; the host sums the 4 partials
per batch and adds the (adjusted) bias once.

Device-side simplifications (exact up to fp error):
  - k bias dropped: it shifts every score in a softmax row by the same
    constant, which cancels.
  - v bias folded into the host-side output bias: sum(P)=1 per row, so
    y = P v + bv and the bv term becomes W_p @ bv_full added once on host.

Per-core kernel (strip-pipelined, strips of 512 queries, s=0..3):
    per strip: k tiles [128,128] (2 heads stacked), q [128,512] (+bias via
    ACT Identity), v tiles [128,256]; then per 128-key round:
      S^T pair [128,1024] = two row-packed K=64 matmuls (heads at array
        rows 0-63 / 64-127) into a 2-bank PSUM tile
      P^T pair = one ACT Exp over the pair tile -> SBUF bf16
        (causal diagonal blocks masked via gpsimd affine_select)
      PV: col-packed pair (M=64+64 into one bank) accumulated over rounds
      denominators: 4 concurrent M=1 matmuls with a ones column into
        PSUM partitions {0,32,64,96}
    normalize via DVE reciprocal + gpsimd partition_broadcast + DVE mult
    projection emits out^T = (wp tile).T @ y strip; host transposes back.
"""
import math
from contextlib import ExitStack

import ml_dtypes
import numpy as np

import concourse.bacc as bacc
import concourse.bass as bass
import concourse.mybir as mybir
import concourse.tile as tile
from concourse.bass_utils import run_bass_kernel_spmd

F32 = mybir.dt.float32
BF16 = mybir.dt.bfloat16
MMDT = BF16                    # dtype for all TensorE-facing tensors

N_CORES = 8
B, T, C, H = 2, 2048, 1024, 16
D = 64
GROUPS = N_CORES // B          # head groups per batch = 4
HPC = H // GROUPS              # heads per core = 4
CS = HPC * D                   # channel slice per core = 256
KT = C // 128                  # contraction tiles over C = 8
NS = T // 512                  # 512-wide query strips = 4
TT = T // 128                  # 128-row key tiles = 16


def build_nc():
    nc = bacc.Bacc("TRN2", target_bir_lowering=False, debug=False,
                   num_devices=N_CORES)

    xT = nc.dram_tensor("xT", [C, T], MMDT, kind="ExternalInput")
    wqkT = nc.dram_tensor("wqkT", [C, 2 * CS], MMDT, kind="ExternalInput")
    bq = nc.dram_tensor("bq", [2, 128, 1], F32, kind="ExternalInput")
    wvT = nc.dram_tensor("wvT", [C, CS], MMDT, kind="ExternalInput")
    wpT = nc.dram_tensor("wpT", [CS, C], MMDT, kind="ExternalInput")
    outT = nc.dram_tensor("outT", [C, T], F32, kind="ExternalOutput")

    xTr = xT.ap().rearrange("(kt p) t -> kt p t", p=128)
    wqkr = wqkT.ap().rearrange("(kt p) n -> kt p n", p=128)
    wvr = wvT.ap().rearrange("(kt p) n -> kt p n", p=128)
    wpr = wpT.ap().rearrange("(kt p) n -> kt p n", p=128)

    scale = 1.0 / math.sqrt(D)

    with tile.TileContext(nc) as tc, ExitStack() as ctx:
        pw = ctx.enter_context(tc.tile_pool(name="pw", bufs=1))
        px = ctx.enter_context(tc.tile_pool(name="px", bufs=1))
        pq = ctx.enter_context(tc.tile_pool(name="pq", bufs=1))
        pk = ctx.enter_context(tc.tile_pool(name="pk", bufs=1))
        pv = ctx.enter_context(tc.tile_pool(name="pv", bufs=1))
        py = ctx.enter_context(tc.tile_pool(name="py", bufs=1))
        ppt = ctx.enter_context(tc.tile_pool(name="ppt", bufs=4))
        pnorm = ctx.enter_context(tc.tile_pool(name="pnorm", bufs=3))
        pout = ctx.enter_context(tc.tile_pool(name="pout", bufs=4))
        # PSUM: 8 banks: sq 2x2 (S pairs + qk/v/proj rotation) + pv 4x1
        psq = ctx.enter_context(tc.tile_pool(name="psq", bufs=2, space="PSUM"))
        ppv = ctx.enter_context(tc.tile_pool(name="ppv", bufs=4, space="PSUM"))

        # ---- input DMA ----
        wqk_sb, wv_sb = [], []
        for k in range(KT):
            wt = pw.tile([128, 2 * CS], MMDT, tag=f"wqk{k}", name=f"wqk{k}")
            nc.gpsimd.dma_start(wt[:], wqkr[k])
            wqk_sb.append(wt)
        for k in range(KT):
            vt = pw.tile([128, CS], MMDT, tag=f"wv{k}", name=f"wv{k}")
            nc.gpsimd.dma_start(vt[:], wvr[k])
            wv_sb.append(vt)
        bq_sb = []
        for m in range(2):
            bt = pw.tile([128, 1], F32, tag=f"bq{m}", name=f"bq{m}")
            nc.gpsimd.dma_start(bt[:], bq.ap()[m])
            bq_sb.append(bt)
        wp_sb = []
        for k2 in range(2):
            pt_ = pw.tile([128, C], MMDT, tag=f"wp{k2}", name=f"wp{k2}")
            nc.gpsimd.dma_start(pt_[:], wpr[k2])
            wp_sb.append(pt_)
        # x: strips 0,1 as [128,512] quarters (early start), 2,3 as halves
        xq = [[None] * 2 for _ in range(KT)]   # [k][s] s in 0,1
        xh = [None] * KT                        # [k] cols 1024:2048
        for k in range(KT):
            for s in range(2):
                t_ = px.tile([128, 512], MMDT, tag=f"xq{k}_{s}",
                             name=f"xq{k}_{s}")
                nc.sync.dma_start(t_[:], xTr[k][:, s * 512:(s + 1) * 512])
                xq[k][s] = t_
        for k in range(KT):
            t_ = px.tile([128, 1024], MMDT, tag=f"xh{k}", name=f"xh{k}")
            nc.scalar.dma_start(t_[:], xTr[k][:, 1024:2048])
            xh[k] = t_

        def x_strip(k, s):
            """AP of x columns [s*512, (s+1)*512) for contraction tile k."""
            if s < 2:
                return xq[k][s][:]
            return xh[k][:, (s - 2) * 512:(s - 1) * 512]

        # persistent SBUF tensors
        q_sb = [pq.tile([128, T], MMDT, tag=f"q{m}", name=f"q{m}")
                for m in range(2)]
        k_sb = [[pk.tile([128, 128], MMDT, tag=f"k{mp}_{n}", name=f"k{mp}_{n}")
                 for n in range(TT)] for mp in range(2)]
        VA = HPC * (D + 1)     # 260: per-head [v(64) | 1.0]
        v_sb = [pv.tile([128, VA], MMDT, tag=f"v{n}", name=f"v{n}")
                for n in range(TT)]
        y_sb = [py.tile([128, T], MMDT, tag=f"y{k2}", name=f"y{k2}")
                for k2 in range(2)]

        for s in range(NS):
            # ---- k production: pair mp holds heads (2mp, 2mp+1) ----
            for mp in range(2):
                ps = psq.tile([128, 1024], F32, tag="sq", name="ps_k")
                for k in range(KT):
                    nc.tensor.matmul(
                        ps[:, 0:512],
                        wqk_sb[k][:, (2 + mp) * 128:(3 + mp) * 128],
                        x_strip(k, s),
                        start=(k == 0), stop=(k == KT - 1),
                    )
                for j in range(4):
                    nc.vector.tensor_copy(k_sb[mp][4 * s + j][:],
                                          ps[:, j * 128:(j + 1) * 128])
            # ---- q production (bias via ACT Identity) ----
            for mp in range(2):
                ps = psq.tile([128, 1024], F32, tag="sq", name="ps_q")
                for k in range(KT):
                    nc.tensor.matmul(
                        ps[:, 0:512],
                        wqk_sb[k][:, mp * 128:(mp + 1) * 128],
                        x_strip(k, s),
                        start=(k == 0), stop=(k == KT - 1),
                    )
                nc.scalar.activation(
                    q_sb[mp][:, s * 512:(s + 1) * 512], ps[:, 0:512],
                    mybir.ActivationFunctionType.Identity, bias=bq_sb[mp][:])
            # ---- v production for key tiles 4s..4s+3 (ones col per head) ----
            for j in range(4):
                n = 4 * s + j
                ps = psq.tile([128, 1024], F32, tag="sq", name="ps_v")
                for k in range(KT):
                    nc.tensor.matmul(
                        ps[:, 0:CS],
                        x_strip(k, s)[:, j * 128:(j + 1) * 128],
                        wv_sb[k][:],
                        start=(k == 0), stop=(k == KT - 1),
                    )
                vgrp = v_sb[n][:].rearrange("p (g e) -> p g e", e=D + 1)
                vsrc = ps[:, 0:CS].rearrange("p (g e) -> p g e", e=D)
                nc.vector.tensor_copy(vgrp[:, :, 0:D], vsrc)
                nc.vector.memset(vgrp[:, :, D:D + 1], 1.0)

            # ---- attention for strip s ----
            nt = 4 * s + 4
            pv_ps = [ppv.tile([D + 1, 512], F32, tag="pv", name=f"pv{h4}")
                     for h4 in range(4)]
            for n in range(nt):
                off = max(0, n - 4 * s) * 128
                pts = []
                for pp in range(2):
                    st = psq.tile([128, 1024], F32, tag="sq", name="st")
                    for r in range(2):
                        nc.tensor.matmul(
                            st[:, r * 512 + off:(r + 1) * 512],
                            k_sb[pp][n][r * 64:(r + 1) * 64, :],
                            q_sb[pp][r * 64:(r + 1) * 64,
                                     s * 512 + off:(s + 1) * 512],
                            start=True, stop=True,
                        )
                    pt = ppt.tile([128, 1024], MMDT, tag="pt", name="pt")
                    if off == 0:
                        nc.scalar.activation(
                            pt[:], st[:],
                            mybir.ActivationFunctionType.Exp, scale=scale)
                    else:
                        for r in range(2):
                            nc.scalar.activation(
                                pt[:, r * 512 + off:(r + 1) * 512],
                                st[:, r * 512 + off:(r + 1) * 512],
                                mybir.ActivationFunctionType.Exp, scale=scale)
                    if n >= 4 * s:
                        # mixed diagonal block: keep where query >= key
                        for r in range(2):
                            nc.gpsimd.affine_select(
                                out=pt[:, r * 512 + off:r * 512 + off + 128],
                                in_=pt[:, r * 512 + off:r * 512 + off + 128],
                                compare_op=mybir.AluOpType.is_ge,
                                fill=0.0, base=0,
                                pattern=[[1, 128]], channel_multiplier=-1)
                    pts.append(pt)
                for pp in range(2):
                    for r in range(2):
                        h4 = 2 * pp + r
                        nc.tensor.matmul(
                            pv_ps[h4][:, off:512],
                            v_sb[n][:, h4 * (D + 1):(h4 + 1) * (D + 1)],
                            pts[pp][:, r * 512 + off:(r + 1) * 512],
                            start=(n == 0), stop=(n == nt - 1),
                        )

            # ---- normalize: y = y_unnorm / denom ----
            for pp in range(2):
                for r in range(2):
                    h4 = 2 * pp + r
                    # custom DVE/gpsimd ops read partition 0 only: stage the
                    # denominator row at base partition 0 before recip/bcast
                    dtmp = pnorm.tile([1, 512], F32, tag="dtmp", name="dtmp")
                    nc.vector.tensor_copy(dtmp[:], pv_ps[h4][D:D + 1, :])
                    rr = pnorm.tile([1, 512], F32, tag="rr", name="rr")
                    nc.vector.reciprocal_approx_fast(rr[:], dtmp[:])
                    rb = pnorm.tile([64, 512], F32, tag="rb", name="rb")
                    nc.gpsimd.partition_broadcast(rb[:], rr[:])
                    nc.vector.tensor_tensor(
                        y_sb[pp][r * 64:(r + 1) * 64, s * 512:(s + 1) * 512],
                        pv_ps[h4][0:D, :], rb[:],
                        op=mybir.AluOpType.mult)

            # ---- projection for this strip: out^T [C, 512] ----
            for ct in range(8):
                ps = psq.tile([128, 1024], F32, tag="sq", name="ps_o")
                for k2 in range(2):
                    nc.tensor.matmul(
                        ps[:, 0:512],
                        wp_sb[k2][:, ct * 128:(ct + 1) * 128],
                        y_sb[k2][:, s * 512:(s + 1) * 512],
                        start=(k2 == 0), stop=(k2 == 1),
                    )
                ot = pout.tile([128, 512], F32, tag="ot", name="ot")
                nc.vector.tensor_copy(ot[:], ps[:, 0:512])
                nc.sync.dma_start(
                    outT.ap()[ct * 128:(ct + 1) * 128,
                              s * 512:(s + 1) * 512],
                    ot[:])

    nc.compile()
    return nc


def make_in_maps(x, W_attn, b_attn, W_proj):
    """Shard full inputs into the 8 per-core input dicts."""
    x = np.asarray(x, dtype=np.float32)
    W_attn = np.asarray(W_attn, dtype=np.float32)
    b_attn = np.asarray(b_attn, dtype=np.float32)
    W_proj = np.asarray(W_proj, dtype=np.float32)
    in_maps = []
    xTb = [np.ascontiguousarray(x[b_].T) for b_ in range(B)]
    for core in range(N_CORES):
        b_ = core // GROUPS
        g = core % GROUPS
        sl = slice(g * CS, (g + 1) * CS)
        wq = W_attn[sl, :]
        wk = W_attn[C + g * CS:C + (g + 1) * CS, :]
        wv = W_attn[2 * C + g * CS:2 * C + (g + 1) * CS, :]
        bqs = b_attn[sl]
        in_maps.append({
            "xT": xTb[b_].astype(ml_dtypes.bfloat16),
            "wqkT": np.ascontiguousarray(
                np.concatenate([wq, wk], 0).T).astype(ml_dtypes.bfloat16),
            "bq": np.ascontiguousarray(bqs.reshape(2, 128, 1)),
            "wvT": np.ascontiguousarray(wv.T).astype(ml_dtypes.bfloat16),
            "wpT": np.ascontiguousarray(
                W_proj[:, g * CS:(g + 1) * CS].T).astype(ml_dtypes.bfloat16),
        })
    return in_maps


_NC = None


def _get_nc():
    global _NC
    if _NC is None:
        _NC = build_nc()
    return _NC


def run(x, W_attn, b_attn, W_proj, b_proj, trace=False):
    nc = _get_nc()
    in_maps = make_in_maps(x, W_attn, b_attn, W_proj)
    res = run_bass_kernel_spmd(nc, in_maps, core_ids=list(range(N_CORES)),
                               trace=trace)
    out = np.zeros((B, T, C), dtype=np.float32)
    for core in range(N_CORES):
        out[core // GROUPS] += res.results[core]["outT"].T
    # b_proj plus the folded-in v bias: y = P v + bv, sum(P)=1 per row
    b_eff = (np.asarray(b_proj, dtype=np.float32)
             + np.asarray(W_proj, dtype=np.float32)
             @ np.asarray(b_attn, dtype=np.float32)[2 * C:3 * C])
    out += b_eff[None, None, :]
    return out, res


def kernel(x, W_attn, b_attn, W_proj, b_proj):
    out, _ = run(x, W_attn, b_attn, W_proj, b_proj, trace=False)
    return out
